# revision 18
# baseline (speedup 1.0000x reference)
"""Trainium2 Bass kernel for nn_Attention (dense_transformer).

Sharding: 8 cores = 2 batches x 4 heads; each core computes one (batch, head)
attention (head/tensor parallel). Host sums the 4 per-head partial output
projections per batch and adds the bias.

Per-core dataflow (v2):
  x_b [256,4096] fp16 -> q = Wq x, k = Wk x (PE fp16, [64,4096], no scale)
                         vT[m,d] = x_chunk^T Wv (PE fp16)
  scores: T = [k;CONST]^T [q;1] per key-block (PE fp16, 65-deep contraction;
          the CONST row injects the schraudolph bias so approx-exp is 1 op)
  exp (per-tile engine/dtype config):
    ACT: e = Exp(T*0.125 + bias) -> fp8 or fp16 (exact)
    DVE: bits = trunc(max(log2e*T, 0)) -> uint8 =bitcast= fp8  (schraudolph)
         or -> int16 =bitcast= fp16
  AV: fp8 pairs: DoubleRow matmul, 2 key blocks/instr, stationary
      [V8|dV8|ones] (V-residual rides in output rows 64:127, denom row 127);
      fp16 blocks: classic matmul, stationary [V|0|ones].
  U = Wo_dup^T O (f32r, 128-deep: Wo rows 64:127 duplicate d0:62 to fold the
      V-residual; row 127=0 kills the denom row). u stored fp16; dnm from
      o_t row 127. Host: out_b = sum_h U/d + b_out.
"""

import numpy as np
import ml_dtypes

import concourse.bass as bass
import concourse.tile as tile
from concourse import bacc, mybir
from concourse.bass_utils import run_bass_kernel_spmd

HEADS = 4
DIM_HEAD = 64
B = 2
C = 256
N = 4096
NCH = 1024                # n-chunk (query) size of the main pipeline
NB = N // 128             # 32 key blocks
NPAIR = NB // 2           # 16 key-block pairs
F32 = mybir.dt.float32
F32R = mybir.dt.float32r
F16 = mybir.dt.float16
F8 = mybir.dt.float8e4    # e4m3
U8 = mybir.dt.uint8
I16 = mybir.dt.int16
e4np = ml_dtypes.float8_e4m3

# ---- exp affine constants ----------------------------------------------
LOG2E = 1.4426950408889634
EXP_BIAS = -2.0           # global, softmax-invariant (keeps e in fp8 range)
SIG8 = 0.0                # schraudolph-fp8 truncation centering
SIG16 = -57.0             # schraudolph-fp16 centering
# const row value (k_sb row 64; q_sb row 64 = 1.0): s' = s_raw + CONST makes
# fp8-schraudolph bits = LOG2E * s' in one mult+max op.
CONST = float(np.float16((56.0 + 8.0 * EXP_BIAS * LOG2E + SIG8) / LOG2E))
ACT_BIAS = EXP_BIAS - CONST / 8.0
S16_MUL = 128.0 * LOG2E
S16_ADD = 15360.0 - 2048.0 * LOG2E - 128.0 * LOG2E * CONST + SIG16

# ---- per-pair class & per-tile engine config ---------------------------
# pair class: '8' = fp8 DoubleRow pair, 'F' = fp16 classic pair
PAIR_CLASS = ['8', 'F', '8', 'F', '8', 'F', '8', 'F',
              '8', 'F', '8', 'F', '8', 'F', '8', 'F']
# tile engine within pair: fp8 pairs -> ACT exact ('a'); fp16 pairs
# alternate (A,S) / (S,S) so ACT:DVE exp load is ~80:48
def tile_mode(j, t):
    if PAIR_CLASS[j] == '8':
        return 'a'
    return 'A' if t == 0 else 'S'

_CACHED_NC = None


def _build_nc() -> bass.Bass:
    nc = bacc.Bacc(None, target_bir_lowering=False, debug=False)

    x = nc.declare_dram_parameter("x", [C, N], F16, isOutput=False)
    wqk = nc.declare_dram_parameter("wqk", [128, 2, 128], F16, isOutput=False)
    wv = nc.declare_dram_parameter("wv", [128, 2, DIM_HEAD], F16, isOutput=False)
    wo = nc.declare_dram_parameter("wo", [128, C], F32, isOutput=False)
    qkc = nc.declare_dram_parameter("qkc", [2, N], F16, isOutput=False)
    u = nc.declare_dram_parameter("u", [C, N], F16, isOutput=True)
    dnm = nc.declare_dram_parameter("dnm", [1, N], F32, isOutput=True)

    with (
        tile.TileContext(nc) as tc,
        tc.tile_pool(name="singles", bufs=1) as singles,
        tc.tile_pool(name="psum", bufs=3, space="PSUM") as psum,
        tc.tile_pool(name="psumO", bufs=1, space="PSUM") as psumO,
        tc.tile_pool(name="esb", bufs=5) as esb,
        tc.tile_pool(name="osb", bufs=2) as osb,
        tc.tile_pool(name="usb", bufs=4) as usb,
    ):
        x0 = singles.tile([128, N], F16)
        x1 = singles.tile([128, N], F16)
        wqk_sb = singles.tile([128, 2, 128], F16)
        wv_sb = singles.tile([128, 2, DIM_HEAD], F16)
        wo_sb = singles.tile([128, C], F32R)
        q_sb = singles.tile([65, N], F16)   # row 64 = 1.0
        k_sb = singles.tile([65, N], F16)   # row 64 = CONST
        # fp8 AV stationary: [key-local, pair, blk-in-pair, V8|dV8|ones]
        vt8 = singles.tile([128, NPAIR, 2, 128], F8)
        # fp16 AV stationary: [key-local, block, V|zeros|ones]
        vt16 = singles.tile([128, NB, 128], F16)
        v_sb = singles.tile([64, N], F16)   # v rows (pre-transpose)
        bias_t = singles.tile([128, 1], F32)

        XCH = N // 4
        for i in range(4):
            xsl = slice(i * XCH, (i + 1) * XCH)
            nc.scalar.dma_start(x0[:, xsl], x[0:128, xsl])
            nc.scalar.dma_start(x1[:, xsl], x[128:256, xsl])
        nc.sync.dma_start(wqk_sb[:], wqk[:])
        nc.sync.dma_start(wv_sb[:], wv[:])
        nc.sync.dma_start(wo_sb[:], wo[:].bitcast(F32R))
        nc.sync.dma_start(q_sb[64:65, :], qkc[0:1, :])
        nc.sync.dma_start(k_sb[64:65, :], qkc[1:2, :])

        nc.vector.memset(bias_t[:], ACT_BIAS)
        # Pool: one-time memsets (its only jobs)
        nc.gpsimd.memset(vt16[:, :, 64:128], 0.0)
        nc.gpsimd.memset(vt8[:, :, :, 64:128], 0.0)
        ones8 = singles.tile([128, 1], F8, tag="ones8")
        nc.vector.memset(ones8[:], 1.0)
        nc.vector.tensor_copy(
            vt8[:, :, :, 127], ones8[:, 0:1].to_broadcast((128, NPAIR, 2)))
        ones16 = singles.tile([128, 1], F16, tag="ones16")
        nc.vector.memset(ones16[:], 1.0)
        nc.vector.tensor_copy(
            vt16[:, :, 127], ones16[:, 0:1].to_broadcast((128, NB)))

        # ---- projections -------------------------------------------------
        def proj_qk(ch):
            sl = slice(ch * 512, (ch + 1) * 512)
            ps = psum.tile([128, 512], F32, tag="t")
            nc.tensor.matmul(ps[:], wqk_sb[:, 0, :], x0[:, sl], start=True,
                             stop=False)
            nc.tensor.matmul(ps[:], wqk_sb[:, 1, :], x1[:, sl], start=False,
                             stop=True)
            nc.vector.tensor_copy(q_sb[0:64, sl], ps[0:64, :])
            nc.vector.tensor_copy(k_sb[0:64, sl], ps[64:128, :])

        def proj_vrow(ch):
            # v in row layout like q/k: one 512-col chunk
            sl = slice(ch * 512, (ch + 1) * 512)
            ps = psum.tile([64, 512], F32, tag="t", name="psv")
            nc.tensor.matmul(ps[:], wv_sb[:, 0, :], x0[:, sl], start=True,
                             stop=False)
            nc.tensor.matmul(ps[:], wv_sb[:, 1, :], x1[:, sl], start=False,
                             stop=True)
            nc.vector.tensor_copy(v_sb[:, sl], ps[:])

        def stage_v(j):
            # DMA-transpose the pair's two key blocks into vt16, then cvt
            for t in range(2):
                mb = 2 * j + t
                msl = slice(mb * 128, (mb + 1) * 128)
                nc.sync.dma_start_transpose(vt16[:, mb, 0:64], v_sb[:, msl])
            if PAIR_CLASS[j] == '8':
                nc.vector.tensor_copy(                            # fp8 cvt
                    vt8[:, j, :, 0:64], vt16[:, 2 * j:2 * j + 2, 0:64])
                nc.vector.tensor_tensor(                          # dV8
                    vt8[:, j, :, 64:127], vt16[:, 2 * j:2 * j + 2, 0:63],
                    vt8[:, j, :, 0:63], mybir.AluOpType.subtract)

        # q-chunks 0,1 (needed by ci=0 scores) up front; the rest of the
        # projections interleave into ci=0's pair loop (see below)
        proj_qk(0)
        proj_qk(1)

        # ---- main loop ---------------------------------------------------
        def emit_av(ps_o, pend, j):
            first = (j == 0)
            last = (j == NPAIR - 1)
            if PAIR_CLASS[j] == '8':
                e8 = pend[0]
                for s in range(NCH // 512):
                    ssl = slice(s * 512, (s + 1) * 512)
                    nc.tensor.matmul(
                        ps_o[:, ssl], vt8[:, j, :, :], e8[:, :, ssl],
                        start=first, stop=last,
                        perf_mode=mybir.MatmulPerfMode.DoubleRow)
            else:
                for t in range(2):
                    e16 = pend[t]
                    for s in range(NCH // 512):
                        ssl = slice(s * 512, (s + 1) * 512)
                        nc.tensor.matmul(
                            ps_o[:, ssl], vt16[:, 2 * j + t, :], e16[:, ssl],
                            start=first and t == 0, stop=last and t == 1)

        def emit_u(o_t, n0):
            # outproj psum tiles ride the scores ring (tag "t") to save banks
            for half in range(2):
                osl = slice(half * 128, (half + 1) * 128)
                u_t = usb.tile([128, NCH], F16)
                for s in range(NCH // 512):
                    ssl = slice(s * 512, (s + 1) * 512)
                    ps_u = psum.tile([128, 512], F32, tag="t")
                    nc.tensor.matmul(ps_u[:], wo_sb[:, osl], o_t[:, ssl],
                                     start=True, stop=True)
                    nc.vector.tensor_copy(u_t[:, ssl], ps_u[:])
                nc.sync.dma_start(u[osl, n0:n0 + NCH], u_t[:])

        pend_u = None
        for ci in range(N // NCH):
            n0 = ci * NCH
            ps_o = psumO.tile([128, NCH], F32)
            if pend_u is not None:
                emit_u(*pend_u)
            pend = {}           # j -> tuple of exp tiles
            for j in range(NPAIR):
                if ci == 0:
                    # interleave remaining projections: k-chunk ch feeds
                    # pairs 2ch..2ch+1; v staged 2+ pairs ahead of its AV
                    if j % 2 == 0 and 2 <= j // 2 + 2 < 8:
                        proj_qk(j // 2 + 2)
                    if j == 0:
                        proj_vrow(0)
                        stage_v(0)
                        stage_v(1)
                    if j % 2 == 0 and j // 2 + 1 < 8:
                        proj_vrow(j // 2 + 1)
                    if j + 2 < NPAIR:
                        stage_v(j + 2)
                cls = PAIR_CLASS[j]
                if cls == '8':
                    e_pair = esb.tile([128, 2, NCH], F8, tag="e8")
                    tiles = (e_pair,)
                else:
                    e16a = esb.tile([128, NCH], F16, tag="e16a", name="e16a")
                    e16b = esb.tile([128, NCH], F16, tag="e16b", name="e16b")
                    tiles = (e16a, e16b)
                for t in range(2):
                    mb = 2 * j + t
                    msl = slice(mb * 128, (mb + 1) * 128)
                    ps_t = psum.tile([128, NCH], F32, tag="t")
                    for s in range(NCH // 512):
                        nsl = slice(n0 + s * 512, n0 + (s + 1) * 512)
                        ssl = slice(s * 512, (s + 1) * 512)
                        nc.tensor.matmul(ps_t[:, ssl], k_sb[:, msl],
                                         q_sb[:, nsl], start=True, stop=True)
                    if t == 0 and j - 3 in pend:
                        emit_av(ps_o, pend.pop(j - 3), j - 3)
                    mode = tile_mode(j, t)
                    if mode == 'd':
                        nc.vector.tensor_scalar(
                            tiles[0][:, t, :].bitcast(U8), ps_t[:], LOG2E,
                            0.0, mybir.AluOpType.mult, mybir.AluOpType.max)
                    elif mode == 'a':
                        nc.scalar.activation(
                            tiles[0][:, t, :], ps_t[:],
                            mybir.ActivationFunctionType.Exp,
                            bias=bias_t[:], scale=0.125)
                    elif mode == 'A':
                        nc.scalar.activation(
                            tiles[t][:], ps_t[:],
                            mybir.ActivationFunctionType.Exp,
                            bias=bias_t[:], scale=0.125)
                    else:  # 'S'
                        nc.vector.tensor_scalar(
                            tiles[t][:].bitcast(I16), ps_t[:], S16_MUL,
                            S16_ADD, mybir.AluOpType.mult,
                            mybir.AluOpType.add)
                pend[j] = tiles
            for j in sorted(pend):
                emit_av(ps_o, pend[j], j)
            o_t = osb.tile([128, NCH], F32R)
            nc.vector.tensor_copy(o_t[:, 0:512], ps_o[:, 0:512])
            nc.vector.tensor_copy(o_t[:, 512:NCH], ps_o[:, 512:NCH])
            nc.sync.dma_start(dnm[0:1, n0:n0 + NCH],
                              o_t[127:128, :].bitcast(F32))
            pend_u = (o_t, n0)
        emit_u(*pend_u)

    nc.compile()
    return nc


def _get_nc() -> bass.Bass:
    global _CACHED_NC
    if _CACHED_NC is None:
        _CACHED_NC = _build_nc()
    return _CACHED_NC


def _stripe_kxm(w: np.ndarray, dtype) -> np.ndarray:
    """[256, M] -> [128, 2, M] k-subtile layout (c = t*128 + p)."""
    return np.ascontiguousarray(w.reshape(2, 128, -1).transpose(1, 0, 2)).astype(dtype)


def make_in_maps(x, w_qkv, w_out):
    x2 = np.ascontiguousarray(x.reshape(B, C, N)).astype(np.float16)
    qkc = np.empty((2, N), dtype=np.float16)
    qkc[0] = 1.0
    qkc[1] = CONST
    in_maps = []
    for core in range(8):
        b, h = divmod(core, HEADS)
        hs = slice(h * DIM_HEAD, (h + 1) * DIM_HEAD)
        wq_ = w_qkv[0 * C:][hs, :].T            # [256, 64] (no scale fold)
        wk_ = w_qkv[1 * C:][hs, :].T
        wv_ = w_qkv[2 * C:][hs, :].T
        wqk_ = np.concatenate([wq_, wk_], axis=1)
        wo_ = np.zeros((128, C), dtype=np.float32)
        wo_[0:64] = w_out[:, hs].T
        wo_[64:127] = w_out[:, hs].T[0:63]
        in_maps.append({
            "x": x2[b],
            "wqk": _stripe_kxm(wqk_, np.float16),
            "wv": _stripe_kxm(wv_, np.float16),
            "wo": wo_,
            "qkc": qkc,
        })
    return in_maps


def combine(results, b_out):
    out = np.zeros((B, C, N), dtype=np.float32)
    for core in range(8):
        b, _h = divmod(core, HEADS)
        r = results[core]
        out[b] += r["u"].astype(np.float32).reshape(C, N) / r["dnm"].reshape(1, N)
    out += b_out.astype(np.float32)[None, :, None]
    return out.reshape(B, C, 64, 64)


def kernel(x, w_qkv, w_out, b_out, _run_kwargs=None):
    nc = _get_nc()
    in_maps = make_in_maps(np.asarray(x), np.asarray(w_qkv), np.asarray(w_out))
    kw = _run_kwargs or {}
    res = run_bass_kernel_spmd(nc, in_maps, list(range(8)), **kw)
    out = combine(res.results, np.asarray(b_out))
    kernel.last_result = res
    return out


# revision 19
# speedup vs baseline: 1.1215x; 1.1215x over previous
"""Trainium2 Bass kernel for nn_Attention (dense_transformer).

Sharding: 8 cores = 2 batches x 4 heads; each core computes one (batch, head)
attention (head/tensor parallel). Host sums the 4 per-head partial output
projections per batch and adds the bias.

Per-core dataflow (v2):
  x_b [256,4096] fp16 -> q = Wq x, k = Wk x (PE fp16, [64,4096], no scale)
                         vT[m,d] = x_chunk^T Wv (PE fp16)
  scores: T = [k;CONST]^T [q;1] per key-block (PE fp16, 65-deep contraction;
          the CONST row injects the schraudolph bias so approx-exp is 1 op)
  exp (per-tile engine/dtype config):
    ACT: e = Exp(T*0.125 + bias) -> fp8 or fp16 (exact)
    DVE: bits = trunc(max(log2e*T, 0)) -> uint8 =bitcast= fp8  (schraudolph)
         or -> int16 =bitcast= fp16
  AV: fp8 pairs: DoubleRow matmul, 2 key blocks/instr, stationary
      [V8|dV8|ones] (V-residual rides in output rows 64:127, denom row 127);
      fp16 blocks: classic matmul, stationary [V|0|ones].
  U = Wo_dup^T O (f32r, 128-deep: Wo rows 64:127 duplicate d0:62 to fold the
      V-residual; row 127=0 kills the denom row). u stored fp16; dnm from
      o_t row 127. Host: out_b = sum_h U/d + b_out.
"""

import numpy as np
import ml_dtypes

import concourse.bass as bass
import concourse.tile as tile
from concourse import bacc, mybir
from concourse.bass_utils import run_bass_kernel_spmd

HEADS = 4
DIM_HEAD = 64
B = 2
C = 256
N = 4096
NCH = 1024                # n-chunk (query) size of the main pipeline
NB = N // 128             # 32 key blocks
NPAIR = NB // 2           # 16 key-block pairs
F32 = mybir.dt.float32
F32R = mybir.dt.float32r
F16 = mybir.dt.float16
F8 = mybir.dt.float8e4    # e4m3
U8 = mybir.dt.uint8
I16 = mybir.dt.int16
e4np = ml_dtypes.float8_e4m3

# ---- exp affine constants ----------------------------------------------
LOG2E = 1.4426950408889634
EXP_BIAS = -2.0           # global, softmax-invariant (keeps e in fp8 range)
SIG8 = 0.0                # schraudolph-fp8 truncation centering
SIG16 = -57.0             # schraudolph-fp16 centering
# const row value (k_sb row 64; q_sb row 64 = 1.0): s' = s_raw + CONST makes
# fp8-schraudolph bits = LOG2E * s' in one mult+max op.
CONST = float(np.float16((56.0 + 8.0 * EXP_BIAS * LOG2E + SIG8) / LOG2E))
ACT_BIAS = EXP_BIAS - CONST / 8.0
S16_MUL = 128.0 * LOG2E
S16_ADD = 15360.0 - 2048.0 * LOG2E - 128.0 * LOG2E * CONST + SIG16

# ---- per-pair class & per-tile engine config ---------------------------
# pair class: '8' = fp8 DoubleRow pair, 'F' = fp16 classic pair
PAIR_CLASS = ['8', 'F', '8', 'F', '8', 'F', '8', 'F',
              '8', 'F', '8', 'F', '8', 'F', '8', 'F']
# tile engine within pair: fp8 pairs -> ACT exact ('a'); fp16 pairs
# alternate (A,S) / (S,S) so ACT:DVE exp load is ~80:48
def tile_mode(j, t):
    if PAIR_CLASS[j] == '8':
        return 'a'
    return 'A' if t == 0 else 'S'

_CACHED_NC = None


def _build_nc() -> bass.Bass:
    nc = bacc.Bacc(None, target_bir_lowering=False, debug=False)

    x = nc.declare_dram_parameter("x", [C, N], F16, isOutput=False)
    wqk = nc.declare_dram_parameter("wqk", [128, 2, 128], F16, isOutput=False)
    wv = nc.declare_dram_parameter("wv", [128, 2, DIM_HEAD], F16, isOutput=False)
    wo = nc.declare_dram_parameter("wo", [128, C], F32, isOutput=False)
    qkc = nc.declare_dram_parameter("qkc", [2, N], F16, isOutput=False)
    u = nc.declare_dram_parameter("u", [C, N], F16, isOutput=True)
    dnm = nc.declare_dram_parameter("dnm", [1, N], F32, isOutput=True)

    with (
        tile.TileContext(nc) as tc,
        tc.tile_pool(name="singles", bufs=1) as singles,
        tc.tile_pool(name="psum", bufs=3, space="PSUM") as psum,
        tc.tile_pool(name="psumO", bufs=1, space="PSUM") as psumO,
        tc.tile_pool(name="esb", bufs=5) as esb,
        tc.tile_pool(name="osb", bufs=2) as osb,
        tc.tile_pool(name="usb", bufs=4) as usb,
    ):
        x0 = singles.tile([128, N], F16)
        x1 = singles.tile([128, N], F16)
        wqk_sb = singles.tile([128, 2, 128], F16)
        wv_sb = singles.tile([128, 2, DIM_HEAD], F16)
        wo_sb = singles.tile([128, C], F32R)
        q_sb = singles.tile([65, N], F16)   # row 64 = 1.0
        k_sb = singles.tile([65, N], F16)   # row 64 = CONST
        # fp8 AV stationary: [key-local, pair, blk-in-pair, V8|dV8|ones]
        vt8 = singles.tile([128, NPAIR, 2, 128], F8)
        # fp16 AV stationary: [key-local, block, V|zeros|ones]
        vt16 = singles.tile([128, NB, 128], F16)
        bias_t = singles.tile([128, 1], F32)

        for lo, hi in [(0, 512), (512, 1024), (1024, 2048),
                       (2048, 3072), (3072, 4096)]:
            xsl = slice(lo, hi)
            nc.scalar.dma_start(x0[:, xsl], x[0:128, xsl])
            nc.scalar.dma_start(x1[:, xsl], x[128:256, xsl])
        nc.sync.dma_start(wqk_sb[:], wqk[:])
        nc.sync.dma_start(wv_sb[:], wv[:])
        nc.sync.dma_start(wo_sb[:], wo[:].bitcast(F32R))
        nc.sync.dma_start(q_sb[64:65, :], qkc[0:1, :])
        nc.sync.dma_start(k_sb[64:65, :], qkc[1:2, :])

        nc.vector.memset(bias_t[:], ACT_BIAS)
        # Pool: one-time memsets (its only jobs)
        nc.gpsimd.memset(vt16[:, :, 64:128], 0.0)
        nc.gpsimd.memset(vt8[:, :, :, 64:128], 0.0)
        ones8 = singles.tile([128, 1], F8, tag="ones8")
        nc.vector.memset(ones8[:], 1.0)
        nc.vector.tensor_copy(
            vt8[:, :, :, 127], ones8[:, 0:1].to_broadcast((128, NPAIR, 2)))
        ones16 = singles.tile([128, 1], F16, tag="ones16")
        nc.vector.memset(ones16[:], 1.0)
        nc.vector.tensor_copy(
            vt16[:, :, 127], ones16[:, 0:1].to_broadcast((128, NB)))

        # ---- projections -------------------------------------------------
        def proj_qk(ch):
            sl = slice(ch * 512, (ch + 1) * 512)
            ps = psum.tile([128, 512], F32, tag="t")
            nc.tensor.matmul(ps[:], wqk_sb[:, 0, :], x0[:, sl], start=True,
                             stop=False)
            nc.tensor.matmul(ps[:], wqk_sb[:, 1, :], x1[:, sl], start=False,
                             stop=True)
            nc.vector.tensor_copy(q_sb[0:64, sl], ps[0:64, :])
            nc.vector.tensor_copy(k_sb[0:64, sl], ps[64:128, :])

        def proj_v(j):
            # one psum tile covers the pair's two key blocks
            ps = psum.tile([128, 2, 64], F32, tag="t")
            for t in range(2):
                bsl = slice(j * 256 + t * 128, j * 256 + t * 128 + 128)
                nc.tensor.matmul(ps[:, t, :], x0[:, bsl], wv_sb[:, 0, :],
                                 start=True, stop=False)
                nc.tensor.matmul(ps[:, t, :], x1[:, bsl], wv_sb[:, 1, :],
                                 start=False, stop=True)
            if PAIR_CLASS[j] == '8':
                nc.vector.tensor_copy(vt8[:, j, :, 0:64], ps[:])  # fp8 cvt
                nc.vector.tensor_tensor(                          # dV8
                    vt8[:, j, :, 64:127], ps[:, :, 0:63],
                    vt8[:, j, :, 0:63], mybir.AluOpType.subtract)
            else:
                nc.vector.tensor_copy(vt16[:, 2 * j:2 * j + 2, 0:64], ps[:])

        # q-chunks 0,1 (needed by ci=0 scores) up front; the rest of the
        # projections interleave into ci=0's pair loop (see below)
        proj_qk(0)
        proj_qk(1)

        # ---- main loop ---------------------------------------------------
        def emit_av(ps_o, pend, j):
            first = (j == 0)
            last = (j == NPAIR - 1)
            if PAIR_CLASS[j] == '8':
                e8 = pend[0]
                for s in range(NCH // 512):
                    ssl = slice(s * 512, (s + 1) * 512)
                    nc.tensor.matmul(
                        ps_o[:, ssl], vt8[:, j, :, :], e8[:, :, ssl],
                        start=first, stop=last,
                        perf_mode=mybir.MatmulPerfMode.DoubleRow)
            else:
                for t in range(2):
                    e16 = pend[t]
                    for s in range(NCH // 512):
                        ssl = slice(s * 512, (s + 1) * 512)
                        nc.tensor.matmul(
                            ps_o[:, ssl], vt16[:, 2 * j + t, :], e16[:, ssl],
                            start=first and t == 0, stop=last and t == 1)

        def emit_u(o_t, n0):
            # outproj psum tiles ride the scores ring (tag "t") to save banks
            for half in range(2):
                osl = slice(half * 128, (half + 1) * 128)
                u_t = usb.tile([128, NCH], F16)
                for s in range(NCH // 512):
                    ssl = slice(s * 512, (s + 1) * 512)
                    ps_u = psum.tile([128, 512], F32, tag="t")
                    nc.tensor.matmul(ps_u[:], wo_sb[:, osl], o_t[:, ssl],
                                     start=True, stop=True)
                    nc.vector.tensor_copy(u_t[:, ssl], ps_u[:])
                nc.sync.dma_start(u[osl, n0:n0 + NCH], u_t[:])

        pend_u = None
        for ci in range(N // NCH):
            n0 = ci * NCH
            ps_o = psumO.tile([128, NCH], F32)
            if pend_u is not None:
                emit_u(*pend_u)
            pend = {}           # j -> tuple of exp tiles
            for j in range(NPAIR):
                if ci == 0:
                    # interleave remaining projections: k-chunk ch feeds
                    # pairs 2ch..2ch+1; v staged 2+ pairs ahead of its AV
                    if j % 2 == 0 and 2 <= j // 2 + 2 < 8:
                        proj_qk(j // 2 + 2)
                    if j < 14:
                        proj_v(j + 2)
                    if j == 0:
                        proj_v(0)
                        proj_v(1)
                cls = PAIR_CLASS[j]
                if cls == '8':
                    e_pair = esb.tile([128, 2, NCH], F8, tag="e8")
                    tiles = (e_pair,)
                else:
                    e16a = esb.tile([128, NCH], F16, tag="e16a", name="e16a")
                    e16b = esb.tile([128, NCH], F16, tag="e16b", name="e16b")
                    tiles = (e16a, e16b)
                for t in range(2):
                    mb = 2 * j + t
                    msl = slice(mb * 128, (mb + 1) * 128)
                    ps_t = psum.tile([128, NCH], F32, tag="t")
                    for s in range(NCH // 512):
                        nsl = slice(n0 + s * 512, n0 + (s + 1) * 512)
                        ssl = slice(s * 512, (s + 1) * 512)
                        nc.tensor.matmul(ps_t[:, ssl], k_sb[:, msl],
                                         q_sb[:, nsl], start=True, stop=True)
                    if t == 0 and j - 3 in pend:
                        emit_av(ps_o, pend.pop(j - 3), j - 3)
                    mode = tile_mode(j, t)
                    if mode == 'd':
                        nc.vector.tensor_scalar(
                            tiles[0][:, t, :].bitcast(U8), ps_t[:], LOG2E,
                            0.0, mybir.AluOpType.mult, mybir.AluOpType.max)
                    elif mode == 'a':
                        nc.scalar.activation(
                            tiles[0][:, t, :], ps_t[:],
                            mybir.ActivationFunctionType.Exp,
                            bias=bias_t[:], scale=0.125)
                    elif mode == 'A':
                        nc.scalar.activation(
                            tiles[t][:], ps_t[:],
                            mybir.ActivationFunctionType.Exp,
                            bias=bias_t[:], scale=0.125)
                    else:  # 'S'
                        nc.vector.tensor_scalar(
                            tiles[t][:].bitcast(I16), ps_t[:], S16_MUL,
                            S16_ADD, mybir.AluOpType.mult,
                            mybir.AluOpType.add)
                pend[j] = tiles
            for j in sorted(pend):
                emit_av(ps_o, pend[j], j)
            o_t = osb.tile([128, NCH], F32R)
            nc.vector.tensor_copy(o_t[:, 0:512], ps_o[:, 0:512])
            nc.vector.tensor_copy(o_t[:, 512:NCH], ps_o[:, 512:NCH])
            nc.sync.dma_start(dnm[0:1, n0:n0 + NCH],
                              o_t[127:128, :].bitcast(F32))
            pend_u = (o_t, n0)
        emit_u(*pend_u)

    nc.compile()
    return nc


def _get_nc() -> bass.Bass:
    global _CACHED_NC
    if _CACHED_NC is None:
        _CACHED_NC = _build_nc()
    return _CACHED_NC


def _stripe_kxm(w: np.ndarray, dtype) -> np.ndarray:
    """[256, M] -> [128, 2, M] k-subtile layout (c = t*128 + p)."""
    return np.ascontiguousarray(w.reshape(2, 128, -1).transpose(1, 0, 2)).astype(dtype)


def make_in_maps(x, w_qkv, w_out):
    x2 = np.ascontiguousarray(x.reshape(B, C, N)).astype(np.float16)
    qkc = np.empty((2, N), dtype=np.float16)
    qkc[0] = 1.0
    qkc[1] = CONST
    in_maps = []
    for core in range(8):
        b, h = divmod(core, HEADS)
        hs = slice(h * DIM_HEAD, (h + 1) * DIM_HEAD)
        wq_ = w_qkv[0 * C:][hs, :].T            # [256, 64] (no scale fold)
        wk_ = w_qkv[1 * C:][hs, :].T
        wv_ = w_qkv[2 * C:][hs, :].T
        wqk_ = np.concatenate([wq_, wk_], axis=1)
        wo_ = np.zeros((128, C), dtype=np.float32)
        wo_[0:64] = w_out[:, hs].T
        wo_[64:127] = w_out[:, hs].T[0:63]
        in_maps.append({
            "x": x2[b],
            "wqk": _stripe_kxm(wqk_, np.float16),
            "wv": _stripe_kxm(wv_, np.float16),
            "wo": wo_,
            "qkc": qkc,
        })
    return in_maps


def combine(results, b_out):
    out = np.zeros((B, C, N), dtype=np.float32)
    for core in range(8):
        b, _h = divmod(core, HEADS)
        r = results[core]
        out[b] += r["u"].astype(np.float32).reshape(C, N) / r["dnm"].reshape(1, N)
    out += b_out.astype(np.float32)[None, :, None]
    return out.reshape(B, C, 64, 64)


def kernel(x, w_qkv, w_out, b_out, _run_kwargs=None):
    nc = _get_nc()
    in_maps = make_in_maps(np.asarray(x), np.asarray(w_qkv), np.asarray(w_out))
    kw = _run_kwargs or {}
    res = run_bass_kernel_spmd(nc, in_maps, list(range(8)), **kw)
    out = combine(res.results, np.asarray(b_out))
    kernel.last_result = res
    return out


# revision 20
# speedup vs baseline: 1.1622x; 1.0363x over previous
"""Trainium2 Bass kernel for nn_Attention (dense_transformer).

Sharding: 8 cores = 2 batches x 4 heads; each core computes one (batch, head)
attention (head/tensor parallel). Host sums the 4 per-head partial output
projections per batch and adds the bias.

Per-core dataflow (v2):
  x_b [256,4096] fp16 -> q = Wq x, k = Wk x (PE fp16, [64,4096], no scale)
                         vT[m,d] = x_chunk^T Wv (PE fp16)
  scores: T = [k;CONST]^T [q;1] per key-block (PE fp16, 65-deep contraction;
          the CONST row injects the schraudolph bias so approx-exp is 1 op)
  exp (per-tile engine/dtype config):
    ACT: e = Exp(T*0.125 + bias) -> fp8 or fp16 (exact)
    DVE: bits = trunc(max(log2e*T, 0)) -> uint8 =bitcast= fp8  (schraudolph)
         or -> int16 =bitcast= fp16
  AV: fp8 pairs: DoubleRow matmul, 2 key blocks/instr, stationary
      [V8|dV8|ones] (V-residual rides in output rows 64:127, denom row 127);
      fp16 blocks: classic matmul, stationary [V|0|ones].
  U = Wo_dup^T O (f32r, 128-deep: Wo rows 64:127 duplicate d0:62 to fold the
      V-residual; row 127=0 kills the denom row). u stored fp16; dnm from
      o_t row 127. Host: out_b = sum_h U/d + b_out.
"""

import numpy as np
import ml_dtypes

import concourse.bass as bass
import concourse.tile as tile
from concourse import bacc, mybir
from concourse.bass_utils import run_bass_kernel_spmd

HEADS = 4
DIM_HEAD = 64
B = 2
C = 256
N = 4096
NCH = 1024                # n-chunk (query) size of the main pipeline
NB = N // 128             # 32 key blocks
NPAIR = NB // 2           # 16 key-block pairs
F32 = mybir.dt.float32
F32R = mybir.dt.float32r
F16 = mybir.dt.float16
F8 = mybir.dt.float8e4    # e4m3
U8 = mybir.dt.uint8
I16 = mybir.dt.int16
e4np = ml_dtypes.float8_e4m3

# ---- exp affine constants ----------------------------------------------
LOG2E = 1.4426950408889634
EXP_BIAS = -2.0           # global, softmax-invariant (keeps e in fp8 range)
SIG8 = 0.0                # schraudolph-fp8 truncation centering
SIG16 = -57.0             # schraudolph-fp16 centering
# const row value (k_sb row 64; q_sb row 64 = 1.0): s' = s_raw + CONST makes
# fp8-schraudolph bits = LOG2E * s' in one mult+max op.
CONST = float(np.float16((56.0 + 8.0 * EXP_BIAS * LOG2E + SIG8) / LOG2E))
ACT_BIAS = EXP_BIAS - CONST / 8.0
S16_MUL = 128.0 * LOG2E
S16_ADD = 15360.0 - 2048.0 * LOG2E - 128.0 * LOG2E * CONST + SIG16

# ---- per-pair class & per-tile engine config ---------------------------
# pair class: '8' = fp8 DoubleRow pair, 'F' = fp16 classic pair
PAIR_CLASS = ['8', 'F', '8', 'F', '8', 'F', '8', 'F',
              '8', 'F', '8', 'F', '8', 'F', '8', 'F']
# tile engine within pair: fp8 pairs -> ACT exact ('a'); fp16 pairs
# alternate (A,S) / (S,S) so ACT:DVE exp load is ~80:48
def tile_mode(j, t):
    if PAIR_CLASS[j] == '8':
        return 'a'
    return 'A' if t == 0 else 'S'

_CACHED_NC = None


def _build_nc() -> bass.Bass:
    nc = bacc.Bacc(None, target_bir_lowering=False, debug=False)

    x = nc.declare_dram_parameter("x", [C, N], F16, isOutput=False)
    wqk = nc.declare_dram_parameter("wqk", [128, 2, 128], F16, isOutput=False)
    wv = nc.declare_dram_parameter("wv", [128, 2, DIM_HEAD], F16, isOutput=False)
    wo = nc.declare_dram_parameter("wo", [128, C], F32, isOutput=False)
    qkc = nc.declare_dram_parameter("qkc", [2, N], F16, isOutput=False)
    u = nc.declare_dram_parameter("u", [C, N], F16, isOutput=True)
    dnm = nc.declare_dram_parameter("dnm", [1, N], F32, isOutput=True)

    with (
        tile.TileContext(nc) as tc,
        tc.tile_pool(name="singles", bufs=1) as singles,
        tc.tile_pool(name="psum", bufs=3, space="PSUM") as psum,
        tc.tile_pool(name="psumO", bufs=1, space="PSUM") as psumO,
        tc.tile_pool(name="esb", bufs=5) as esb,
        tc.tile_pool(name="osb", bufs=2) as osb,
        tc.tile_pool(name="usb", bufs=4) as usb,
    ):
        x0 = singles.tile([128, N], F16)
        x1 = singles.tile([128, N], F16)
        wqk_sb = singles.tile([128, 2, 128], F16)
        wv_sb = singles.tile([128, 2, DIM_HEAD], F16)
        wo_sb = singles.tile([128, C], F32R)
        q_sb = singles.tile([65, N], F16)   # row 64 = 1.0
        k_sb = singles.tile([65, N], F16)   # row 64 = CONST
        # fp8 AV stationary: [key-local, pair, blk-in-pair, V8|dV8|ones]
        vt8 = singles.tile([128, NPAIR, 2, 128], F8)
        # fp16 AV stationary: [key-local, block, V|zeros|ones]
        vt16 = singles.tile([128, NB, 128], F16)
        bias_t = singles.tile([128, 1], F32)

        XCH = N // 4
        for i in range(4):
            xsl = slice(i * XCH, (i + 1) * XCH)
            nc.scalar.dma_start(x0[:, xsl], x[0:128, xsl])
            nc.scalar.dma_start(x1[:, xsl], x[128:256, xsl])
        nc.sync.dma_start(wqk_sb[:], wqk[:])
        nc.sync.dma_start(wv_sb[:], wv[:])
        nc.sync.dma_start(wo_sb[:], wo[:].bitcast(F32R))
        nc.sync.dma_start(q_sb[64:65, :], qkc[0:1, :])
        nc.sync.dma_start(k_sb[64:65, :], qkc[1:2, :])

        nc.vector.memset(bias_t[:], ACT_BIAS)
        # Pool: one-time memsets (its only jobs)
        nc.gpsimd.memset(vt16[:, :, 64:128], 0.0)
        nc.gpsimd.memset(vt8[:, :, :, 64:128], 0.0)
        ones8 = singles.tile([128, 1], F8, tag="ones8")
        nc.vector.memset(ones8[:], 1.0)
        nc.vector.tensor_copy(
            vt8[:, :, :, 127], ones8[:, 0:1].to_broadcast((128, NPAIR, 2)))
        ones16 = singles.tile([128, 1], F16, tag="ones16")
        nc.vector.memset(ones16[:], 1.0)
        nc.vector.tensor_copy(
            vt16[:, :, 127], ones16[:, 0:1].to_broadcast((128, NB)))

        # ---- projections -------------------------------------------------
        def proj_qk(ch):
            sl = slice(ch * 512, (ch + 1) * 512)
            ps = psum.tile([128, 512], F32, tag="t")
            nc.tensor.matmul(ps[:], wqk_sb[:, 0, :], x0[:, sl], start=True,
                             stop=False)
            nc.tensor.matmul(ps[:], wqk_sb[:, 1, :], x1[:, sl], start=False,
                             stop=True)
            nc.vector.tensor_copy(q_sb[0:64, sl], ps[0:64, :])
            nc.vector.tensor_copy(k_sb[0:64, sl], ps[64:128, :])

        def proj_v(j):
            # one psum tile covers the pair's two key blocks
            ps = psum.tile([128, 2, 64], F32, tag="t")
            for t in range(2):
                bsl = slice(j * 256 + t * 128, j * 256 + t * 128 + 128)
                nc.tensor.matmul(ps[:, t, :], x0[:, bsl], wv_sb[:, 0, :],
                                 start=True, stop=False)
                nc.tensor.matmul(ps[:, t, :], x1[:, bsl], wv_sb[:, 1, :],
                                 start=False, stop=True)
            if PAIR_CLASS[j] == '8':
                nc.vector.tensor_copy(vt8[:, j, :, 0:64], ps[:])  # fp8 cvt
                nc.vector.tensor_tensor(                          # dV8
                    vt8[:, j, :, 64:127], ps[:, :, 0:63],
                    vt8[:, j, :, 0:63], mybir.AluOpType.subtract)
            else:
                nc.vector.tensor_copy(vt16[:, 2 * j:2 * j + 2, 0:64], ps[:])

        # q-chunks 0,1 (needed by ci=0 scores) up front; the rest of the
        # projections interleave into ci=0's pair loop (see below)
        proj_qk(0)
        proj_qk(1)

        # ---- main loop ---------------------------------------------------
        def emit_av(ps_o, pend, j):
            first = (j == 0)
            last = (j == NPAIR - 1)
            if PAIR_CLASS[j] == '8':
                e8 = pend[0]
                for s in range(NCH // 512):
                    ssl = slice(s * 512, (s + 1) * 512)
                    nc.tensor.matmul(
                        ps_o[:, ssl], vt8[:, j, :, :], e8[:, :, ssl],
                        start=first, stop=last,
                        perf_mode=mybir.MatmulPerfMode.DoubleRow)
            else:
                for t in range(2):
                    e16 = pend[t]
                    for s in range(NCH // 512):
                        ssl = slice(s * 512, (s + 1) * 512)
                        nc.tensor.matmul(
                            ps_o[:, ssl], vt16[:, 2 * j + t, :], e16[:, ssl],
                            start=first and t == 0, stop=last and t == 1)

        def emit_u(o_t, n0):
            # outproj psum tiles ride the scores ring (tag "t") to save banks
            for half in range(2):
                osl = slice(half * 128, (half + 1) * 128)
                u_t = usb.tile([128, NCH], F16)
                for s in range(NCH // 512):
                    ssl = slice(s * 512, (s + 1) * 512)
                    ps_u = psum.tile([128, 512], F32, tag="t")
                    nc.tensor.matmul(ps_u[:], wo_sb[:, osl], o_t[:, ssl],
                                     start=True, stop=True)
                    nc.vector.tensor_copy(u_t[:, ssl], ps_u[:])
                nc.sync.dma_start(u[osl, n0:n0 + NCH], u_t[:])

        pend_u = None
        for ci in range(N // NCH):
            n0 = ci * NCH
            ps_o = psumO.tile([128, NCH], F32)
            if pend_u is not None:
                emit_u(*pend_u)
            pend = {}           # j -> tuple of exp tiles
            for j in range(NPAIR):
                if ci == 0:
                    # interleave remaining projections: k-chunk ch feeds
                    # pairs 2ch..2ch+1; v staged 2+ pairs ahead of its AV
                    if j % 2 == 0 and 2 <= j // 2 + 2 < 8:
                        proj_qk(j // 2 + 2)
                    if j < 14:
                        proj_v(j + 2)
                    if j == 0:
                        proj_v(0)
                        proj_v(1)
                cls = PAIR_CLASS[j]
                if cls == '8':
                    e_pair = esb.tile([128, 2, NCH], F8, tag="e8")
                    tiles = (e_pair,)
                else:
                    e16a = esb.tile([128, NCH], F16, tag="e16a", name="e16a")
                    e16b = esb.tile([128, NCH], F16, tag="e16b", name="e16b")
                    tiles = (e16a, e16b)
                for t in range(2):
                    mb = 2 * j + t
                    msl = slice(mb * 128, (mb + 1) * 128)
                    ps_t = psum.tile([128, NCH], F32, tag="t")
                    for s in range(NCH // 512):
                        nsl = slice(n0 + s * 512, n0 + (s + 1) * 512)
                        ssl = slice(s * 512, (s + 1) * 512)
                        nc.tensor.matmul(ps_t[:, ssl], k_sb[:, msl],
                                         q_sb[:, nsl], start=True, stop=True)
                    if t == 0 and j - 3 in pend:
                        emit_av(ps_o, pend.pop(j - 3), j - 3)
                    mode = tile_mode(j, t)
                    if mode == 'd':
                        nc.vector.tensor_scalar(
                            tiles[0][:, t, :].bitcast(U8), ps_t[:], LOG2E,
                            0.0, mybir.AluOpType.mult, mybir.AluOpType.max)
                    elif mode == 'a':
                        nc.scalar.activation(
                            tiles[0][:, t, :], ps_t[:],
                            mybir.ActivationFunctionType.Exp,
                            bias=bias_t[:], scale=0.125)
                    elif mode == 'A':
                        nc.scalar.activation(
                            tiles[t][:], ps_t[:],
                            mybir.ActivationFunctionType.Exp,
                            bias=bias_t[:], scale=0.125)
                    else:  # 'S'
                        nc.vector.tensor_scalar(
                            tiles[t][:].bitcast(I16), ps_t[:], S16_MUL,
                            S16_ADD, mybir.AluOpType.mult,
                            mybir.AluOpType.add)
                pend[j] = tiles
            for j in sorted(pend):
                emit_av(ps_o, pend[j], j)
            o_t = osb.tile([128, NCH], F32R)
            nc.vector.tensor_copy(o_t[:, 0:512], ps_o[:, 0:512])
            nc.vector.tensor_copy(o_t[:, 512:NCH], ps_o[:, 512:NCH])
            nc.sync.dma_start(dnm[0:1, n0:n0 + NCH],
                              o_t[127:128, :].bitcast(F32))
            pend_u = (o_t, n0)
        emit_u(*pend_u)

    nc.compile()
    return nc


def _get_nc() -> bass.Bass:
    global _CACHED_NC
    if _CACHED_NC is None:
        _CACHED_NC = _build_nc()
    return _CACHED_NC


def _stripe_kxm(w: np.ndarray, dtype) -> np.ndarray:
    """[256, M] -> [128, 2, M] k-subtile layout (c = t*128 + p)."""
    return np.ascontiguousarray(w.reshape(2, 128, -1).transpose(1, 0, 2)).astype(dtype)


def make_in_maps(x, w_qkv, w_out):
    x2 = np.ascontiguousarray(x.reshape(B, C, N)).astype(np.float16)
    qkc = np.empty((2, N), dtype=np.float16)
    qkc[0] = 1.0
    qkc[1] = CONST
    in_maps = []
    for core in range(8):
        b, h = divmod(core, HEADS)
        hs = slice(h * DIM_HEAD, (h + 1) * DIM_HEAD)
        wq_ = w_qkv[0 * C:][hs, :].T            # [256, 64] (no scale fold)
        wk_ = w_qkv[1 * C:][hs, :].T
        wv_ = w_qkv[2 * C:][hs, :].T
        wqk_ = np.concatenate([wq_, wk_], axis=1)
        wo_ = np.zeros((128, C), dtype=np.float32)
        wo_[0:64] = w_out[:, hs].T
        wo_[64:127] = w_out[:, hs].T[0:63]
        in_maps.append({
            "x": x2[b],
            "wqk": _stripe_kxm(wqk_, np.float16),
            "wv": _stripe_kxm(wv_, np.float16),
            "wo": wo_,
            "qkc": qkc,
        })
    return in_maps


def combine(results, b_out):
    out = np.zeros((B, C, N), dtype=np.float32)
    for core in range(8):
        b, _h = divmod(core, HEADS)
        r = results[core]
        out[b] += r["u"].astype(np.float32).reshape(C, N) / r["dnm"].reshape(1, N)
    out += b_out.astype(np.float32)[None, :, None]
    return out.reshape(B, C, 64, 64)


def kernel(x, w_qkv, w_out, b_out, _run_kwargs=None):
    nc = _get_nc()
    in_maps = make_in_maps(np.asarray(x), np.asarray(w_qkv), np.asarray(w_out))
    kw = _run_kwargs or {}
    res = run_bass_kernel_spmd(nc, in_maps, list(range(8)), **kw)
    out = combine(res.results, np.asarray(b_out))
    kernel.last_result = res
    return out
